# revision 8
# baseline (speedup 1.0000x reference)
"""Trainium2 Bass kernel for nn_MetaEmbedding_Classifier (retrieval_knn).

Data-parallel over batch B=1024 across 8 NeuronCores (128 rows/core).
All relation/centroid/classifier weights replicated per core.

Per-core pipeline (all math on device; host only slices / transposes /
pads layouts):
  feat  = concat(head, tail) @ fc2_w.T + fc2_b          (PE, K-chunked)
  featT = feat^T (PE transposes, stationary operand for later matmuls)
  negd2 = 2*(x @ centT  +  ones (x) (-c2/2)) - |x|^2    (PE + DVE)
  values/labels = max / max_index over negd2             (DVE top-8 ops)
  att   = softmax(x @ all_relation.T) masked at `relation`
  mf    = att^T.T @ centroids                            (PE)
  p     = sigmoid(x @ sel_w.T + sel_b)
  fused = (mf - feat)*p + feat
  logits = (ex @ cls_w.T) * (SCALE/||cls_w||)  with ex = fused/(1+|fused|)

K-chunked operands are packed host-side into [128, nchunk*N] so every DMA
is a single per-partition-contiguous transfer.
"""

from contextlib import ExitStack

import numpy as np

import concourse.bass as bass
import concourse.mybir as mybir
import concourse.tile as tile
from concourse import bacc
from concourse.bass_utils import run_bass_kernel_spmd

B, D, R = 1024, 512, 500
NCORES = 8
BC = B // NCORES  # 128 batch rows per core
KD = D // 128     # 4 k-chunks over D
K2D = 2 * D // 128  # 8 k-chunks over 2D
RCH = 125         # att/cent r-chunk (4 * 125 = 500)
SCALE = 16.0
EPS = 1e-12

F32 = mybir.dt.float32
F32R = mybir.dt.float32r
I32 = mybir.dt.int32
U32 = mybir.dt.uint32

# f32r = fp32 matmul mode that streams 1 row/cycle (vs 4 for plain fp32)
# when the moving free dim >= 256.
USE_F32R = False


def _r(ap):
    """AP used as a matmul operand: bitcast to float32r for 4x PE rate."""
    return ap.bitcast(F32R) if USE_F32R else ap


def _pack(mat, p):
    """[K*p, N] -> [p, K*N] so chunk k is columns k*N:(k+1)*N."""
    kp, n = mat.shape
    k = kp // p
    return np.ascontiguousarray(
        mat.reshape(k, p, n).transpose(1, 0, 2).reshape(p, k * n)
    )


def build_program():
    nc = bacc.Bacc(target_bir_lowering=False, trn_type="TRN2", debug=False)

    din = {}

    def inp(name, shape):
        din[name] = nc.dram_tensor(name, shape, F32, kind="ExternalInput").ap()

    inp("xT", [128, K2D * BC])       # concat(head,tail)^T packed, per core
    inp("w2T", [128, K2D * D])       # fc2_w.T packed (replicated)
    inp("b2row", [1, D])             # fc2_b row
    inp("centT", [128, KD * R])      # centroids[3:].T zero-padded, packed
    inp("cent", [RCH, 4 * D])        # centroids row-major, packed in 4 r-chunks
    inp("relT", [128, KD * R])       # all_relation.T packed
    inp("clsT", [128, KD * R])       # cls_w.T packed
    inp("selT", [128, KD])           # sel_w.T packed
    inp("selb", [1, 1])
    inp("iota", [1, R])              # arange(500) as f32
    inp("relf", [BC, 1])             # relation indices as f32, per core
    inp("ident", [128, 128])         # identity for PE transposes

    dout = {
        "logits": nc.dram_tensor("logits", [BC, R], F32,
                                 kind="ExternalOutput").ap(),
        "feat": nc.dram_tensor("feat", [BC, D], F32,
                               kind="ExternalOutput").ap(),
        "values": nc.dram_tensor("values", [BC, 1], F32,
                                 kind="ExternalOutput").ap(),
        "labels": nc.dram_tensor("labels", [BC, 1], I32,
                                 kind="ExternalOutput").ap(),
    }

    with tile.TileContext(nc) as tc:
        with ExitStack() as ctx:
            _build(ctx, tc, din, dout)

    nc.compile()
    return nc


def _build(ctx, tc, din, dout):
    nc = tc.nc
    Exp = mybir.ActivationFunctionType.Exp
    Sqrt = mybir.ActivationFunctionType.Sqrt
    Square = mybir.ActivationFunctionType.Square
    Sigmoid = mybir.ActivationFunctionType.Sigmoid
    AX = mybir.AxisListType.X
    Op = mybir.AluOpType

    singles = ctx.enter_context(tc.tile_pool(name="singles", bufs=1))
    work = ctx.enter_context(tc.tile_pool(name="work", bufs=2))
    psmm = ctx.enter_context(tc.tile_pool(name="psmm", bufs=3, space="PSUM"))
    pstr = ctx.enter_context(tc.tile_pool(name="pstr", bufs=2, space="PSUM"))
    psrow = ctx.enter_context(tc.tile_pool(name="psrow", bufs=1, space="PSUM"))

    def load(name, shape, pool=singles):
        t = pool.tile(shape, F32, tag=name, name=name + "_sb")
        nc.sync.dma_start(out=t[:], in_=din[name])
        return t

    # ---- resident constants / weights ----
    ident = load("ident", [128, 128])
    iota = load("iota", [1, R])
    b2row = load("b2row", [1, D])
    selb = load("selb", [1, 1])
    relf = load("relf", [BC, 1])
    selT_f = load("selT", [128, KD])
    xT_f = load("xT", [128, K2D * BC])
    w2T_f = load("w2T", [128, K2D * D])
    centT_f = load("centT", [128, KD * R])
    relT_f = load("relT", [128, KD * R])
    clsT_f = load("clsT", [128, KD * R])
    cent_f = load("cent", [RCH, 4 * D])

    xT = [xT_f[:, k * BC:(k + 1) * BC] for k in range(K2D)]
    w2T = [w2T_f[:, k * D:(k + 1) * D] for k in range(K2D)]
    centT = [centT_f[:, k * R:(k + 1) * R] for k in range(KD)]
    relT = [relT_f[:, k * R:(k + 1) * R] for k in range(KD)]
    clsT = [clsT_f[:, k * R:(k + 1) * R] for k in range(KD)]
    selT = [selT_f[:, k:k + 1] for k in range(KD)]
    cent = [cent_f[:, j * D:(j + 1) * D] for j in range(4)]

    ones = singles.tile([128, 128], F32, tag="ones", name="ones")
    nc.vector.memset(ones[:], 1.0)
    ones_row = ones[0:1, :]   # [1, 128] lhsT for K=1 row-broadcast matmuls
    ones_col = ones[:, 0:1]   # [128, 1] lhsT for partition-sum matmuls

    # ================= fc2: feat = concat @ fc2_w.T + b =================
    ps_feat = psmm.tile([BC, D], F32, tag="mm", name="ps_feat")
    for k in range(K2D):
        nc.tensor.matmul(ps_feat[:], _r(xT[k]), _r(w2T[k]), start=(k == 0),
                         stop=False)
    nc.tensor.matmul(ps_feat[:], _r(ones_row), _r(b2row[:]), start=False,
                     stop=True)

    feat = singles.tile([BC, D], F32, tag="feat", name="feat")
    nc.vector.tensor_copy(feat[:], ps_feat[:])
    nc.sync.dma_start(out=dout["feat"], in_=feat[:])

    # |x|^2 per row (ACT square w/ fused free-dim accumulation)
    x2 = singles.tile([BC, 1], F32, tag="x2", name="x2")
    sq_d = work.tile([BC, D], F32, tag="sq_d", name="sq_d0")
    nc.scalar.activation(sq_d[:], ps_feat[:], Square, accum_out=x2[:])

    # featT chunks [128, BC] (stationary operand for x @ W.T style matmuls)
    featT = []
    for k in range(KD):
        pt = pstr.tile([128, BC], F32, tag="tr", name=f"ps_ft{k}")
        nc.tensor.transpose(pt[:], feat[:, k * 128:(k + 1) * 128], ident[:])
        st = singles.tile([128, BC], F32, tag=f"featT{k}", name=f"featT{k}")
        nc.vector.tensor_copy(st[:], pt[:])
        featT.append(st)

    # ================= distances to centroids[3:] =================
    # c2m = -0.5 * sum_d centT^2 (per padded column; pad cols give 0)
    ps_c2 = psrow.tile([1, R], F32, tag="row", name="ps_c2")
    for k in range(KD):
        sq = work.tile([128, R], F32, tag="sq_r", name=f"sq_c{k}")
        nc.scalar.activation(sq[:], centT[k], Square)
        nc.tensor.matmul(ps_c2[:], _r(ones_col), _r(sq[:]), start=(k == 0),
                         stop=(k == KD - 1))
    c2m = singles.tile([1, R], F32, tag="c2m", name="c2m")
    nc.vector.tensor_scalar(c2m[:], ps_c2[:], -0.5, None, op0=Op.mult)

    # G = x @ centT - c2/2  ->  negd2 = 2G - |x|^2
    ps_g = psmm.tile([BC, R], F32, tag="mm", name="ps_g")
    for k in range(KD):
        nc.tensor.matmul(ps_g[:], _r(featT[k][:]), _r(centT[k]), start=(k == 0),
                         stop=False)
    nc.tensor.matmul(ps_g[:], _r(ones_row), _r(c2m[:]), start=False, stop=True)

    negd2 = work.tile([BC, R], F32, tag="negd2", name="negd2")
    nc.vector.tensor_scalar(negd2[:], ps_g[:], 2.0, x2[:], op0=Op.mult,
                            op1=Op.subtract)

    # top-1 via DVE top-8 ops over the 497 valid columns
    maxv = work.tile([BC, 8], F32, tag="maxv", name="maxv")
    nc.vector.max(maxv[:], negd2[:, 0:R - 3])
    idx = work.tile([BC, 8], U32, tag="idx", name="idx")
    nc.vector.max_index(idx[:], maxv[:], negd2[:, 0:R - 3])

    vtmp = work.tile([BC, 1], F32, tag="vtmp", name="vtmp")
    nc.vector.tensor_scalar(vtmp[:], maxv[:, 0:1], -1.0, EPS, op0=Op.mult,
                            op1=Op.max)
    values = work.tile([BC, 1], F32, tag="values", name="values_sb")
    nc.scalar.activation(values[:], vtmp[:], Sqrt)
    nc.sync.dma_start(out=dout["values"], in_=values[:])

    idxf = work.tile([BC, 1], F32, tag="idxf", name="idxf")
    nc.vector.tensor_copy(idxf[:], idx[:, 0:1])
    labels = work.tile([BC, 1], I32, tag="labels", name="labels_sb")
    nc.vector.tensor_scalar(labels[:], idxf[:], 3.0, None, op0=Op.add)
    nc.sync.dma_start(out=dout["labels"], in_=labels[:])

    # ================= masked softmax attention =================
    ps_s = psmm.tile([BC, R], F32, tag="mm", name="ps_s")
    for k in range(KD):
        nc.tensor.matmul(ps_s[:], _r(featT[k][:]), _r(relT[k]), start=(k == 0),
                         stop=(k == KD - 1))

    smax = work.tile([BC, 1], F32, tag="smax", name="smax")
    nc.vector.tensor_reduce(smax[:], ps_s[:], axis=AX, op=Op.max)
    nsmax = work.tile([BC, 1], F32, tag="nsmax", name="nsmax")
    nc.vector.tensor_scalar(nsmax[:], smax[:], -1.0, None, op0=Op.mult)

    e = work.tile([BC, R], F32, tag="e", name="e")
    ssum = work.tile([BC, 1], F32, tag="ssum", name="ssum")
    nc.scalar.activation(e[:], ps_s[:], Exp, bias=nsmax[:], accum_out=ssum[:])
    rinv = work.tile([BC, 1], F32, tag="rinv", name="rinv")
    nc.vector.reciprocal(rinv[:], ssum[:])

    # mask = (iota != relation); iota broadcast across partitions via K=1 matmul
    ps_i = psmm.tile([BC, R], F32, tag="mm", name="ps_i")
    nc.tensor.matmul(ps_i[:], _r(ones_row), _r(iota[:]), start=True, stop=True)
    mask = work.tile([BC, R], F32, tag="mask", name="mask")
    nc.vector.tensor_scalar(mask[:], ps_i[:], relf[:], None, op0=Op.not_equal)

    am = work.tile([BC, R], F32, tag="am", name="am")
    nc.vector.scalar_tensor_tensor(am[:], e[:], rinv[:], mask[:], op0=Op.mult,
                                   op1=Op.mult)

    # attT chunks [125, BC]; mf = att @ centroids
    ps_mf = psmm.tile([BC, D], F32, tag="mm", name="ps_mf")
    for j in range(4):
        pt = pstr.tile([RCH, BC], F32, tag="tr", name=f"ps_at{j}")
        nc.tensor.transpose(pt[:], am[:, j * RCH:(j + 1) * RCH], ident[:])
        attT = work.tile([RCH, BC], F32, tag=f"attT{j}", name=f"attT{j}")
        nc.vector.tensor_copy(attT[:], pt[:])
        nc.tensor.matmul(ps_mf[:], _r(attT[:]), _r(cent[j]), start=(j == 0),
                         stop=(j == 3))

    # ================= gate + fuse + cosnorm classifier =================
    ps_sel = psmm.tile([BC, 1], F32, tag="sel", name="ps_sel", bufs=1)
    for k in range(KD):
        nc.tensor.matmul(ps_sel[:], _r(featT[k][:]), _r(selT[k]), start=(k == 0),
                         stop=False)
    nc.tensor.matmul(ps_sel[:], _r(ones_row), _r(selb[:]), start=False,
                     stop=True)
    p = work.tile([BC, 1], F32, tag="p", name="p")
    nc.scalar.activation(p[:], ps_sel[:], Sigmoid)

    # fused = (mf - feat) * p + feat
    dmf = work.tile([BC, D], F32, tag="dmf", name="dmf")
    nc.vector.tensor_tensor(dmf[:], ps_mf[:], feat[:], op=Op.subtract)
    fused = work.tile([BC, D], F32, tag="fused", name="fused")
    nc.vector.scalar_tensor_tensor(fused[:], dmf[:], p[:], feat[:], op0=Op.mult,
                                   op1=Op.add)

    # ex = fused / (1 + |fused|)
    fn2 = work.tile([BC, 1], F32, tag="fn2", name="fn2")
    sq_f = work.tile([BC, D], F32, tag="sq_d", name="sq_d1")
    nc.scalar.activation(sq_f[:], fused[:], Square, accum_out=fn2[:])
    fn = work.tile([BC, 1], F32, tag="fn", name="fn")
    nc.scalar.activation(fn[:], fn2[:], Sqrt)
    fn1 = work.tile([BC, 1], F32, tag="fn1", name="fn1")
    nc.vector.tensor_scalar(fn1[:], fn[:], 1.0, None, op0=Op.add)
    s = work.tile([BC, 1], F32, tag="s", name="s")
    nc.vector.reciprocal(s[:], fn1[:])
    ex = work.tile([BC, D], F32, tag="ex", name="ex")
    nc.vector.tensor_scalar(ex[:], fused[:], s[:], None, op0=Op.mult)

    exT = []
    for k in range(KD):
        pt = pstr.tile([128, BC], F32, tag="tr", name=f"ps_xt{k}")
        nc.tensor.transpose(pt[:], ex[:, k * 128:(k + 1) * 128], ident[:])
        st = work.tile([128, BC], F32, tag=f"exT{k}", name=f"exT{k}")
        nc.vector.tensor_copy(st[:], pt[:])
        exT.append(st)

    # rn = SCALE / ||cls_w|| per class (row vector)
    ps_n2 = psrow.tile([1, R], F32, tag="row", name="ps_n2")
    for k in range(KD):
        sq = work.tile([128, R], F32, tag="sq_r", name=f"sq_n{k}")
        nc.scalar.activation(sq[:], clsT[k], Square)
        nc.tensor.matmul(ps_n2[:], _r(ones_col), _r(sq[:]), start=(k == 0),
                         stop=(k == KD - 1))
    nrm = work.tile([1, R], F32, tag="nrm", name="nrm")
    nc.scalar.activation(nrm[:], ps_n2[:], Sqrt)
    rn0 = work.tile([1, R], F32, tag="rn0", name="rn0")
    nc.vector.reciprocal(rn0[:], nrm[:])
    rn = singles.tile([1, R], F32, tag="rn", name="rn")
    nc.vector.tensor_scalar(rn[:], rn0[:], SCALE, None, op0=Op.mult)

    # logits = (ex @ clsT) * rn  (rn broadcast via K=1 matmul)
    ps_g2 = psmm.tile([BC, R], F32, tag="mm", name="ps_g2")
    for k in range(KD):
        nc.tensor.matmul(ps_g2[:], _r(exT[k][:]), _r(clsT[k]), start=(k == 0),
                         stop=(k == KD - 1))
    ps_rb = psmm.tile([BC, R], F32, tag="mm", name="ps_rb")
    nc.tensor.matmul(ps_rb[:], _r(ones_row), _r(rn[:]), start=True, stop=True)
    rb = work.tile([BC, R], F32, tag="rb", name="rb")
    nc.vector.tensor_copy(rb[:], ps_rb[:])
    logits = work.tile([BC, R], F32, tag="logits", name="logits_sb")
    nc.vector.tensor_tensor(logits[:], ps_g2[:], rb[:], op=Op.mult)
    nc.sync.dma_start(out=dout["logits"], in_=logits[:])


# ---------------------------------------------------------------------------
# host-side entry point
# ---------------------------------------------------------------------------

_CACHE = {}


def prep_in_maps(inputs):
    head = np.asarray(inputs["head_entity"], np.float32)
    tail = np.asarray(inputs["tail_entity"], np.float32)
    relation = np.asarray(inputs["relation"])
    all_rel = np.asarray(inputs["all_relation"], np.float32)
    centroids = np.asarray(inputs["centroids"], np.float32)
    fc2_w = np.asarray(inputs["fc2_w"], np.float32)
    fc2_b = np.asarray(inputs["fc2_b"], np.float32)
    sel_w = np.asarray(inputs["sel_w"], np.float32)
    sel_b = np.asarray(inputs["sel_b"], np.float32)
    cls_w = np.asarray(inputs["cls_w"], np.float32)

    concat = np.concatenate([head, tail], axis=1)          # [B, 2D]
    centT_pad = np.zeros((D, R), np.float32)
    centT_pad[:, : R - 3] = centroids[3:].T

    shared = {
        "w2T": _pack(np.ascontiguousarray(fc2_w.T), 128),  # [128, 8*D]
        "b2row": fc2_b.reshape(1, D),
        "centT": _pack(centT_pad, 128),
        "cent": _pack(centroids, RCH),
        "relT": _pack(np.ascontiguousarray(all_rel.T), 128),
        "clsT": _pack(np.ascontiguousarray(cls_w.T), 128),
        "selT": _pack(np.ascontiguousarray(sel_w.T), 128),
        "selb": sel_b.reshape(1, 1).astype(np.float32),
        "iota": np.arange(R, dtype=np.float32).reshape(1, R),
        "ident": np.eye(128, dtype=np.float32),
    }
    in_maps = []
    for c in range(NCORES):
        sl = slice(c * BC, (c + 1) * BC)
        m = dict(shared)
        m["xT"] = _pack(np.ascontiguousarray(concat[sl].T), 128)
        m["relf"] = relation[sl].astype(np.float32).reshape(BC, 1)
        in_maps.append(m)
    return in_maps


def kernel(**inputs):
    if "nc" not in _CACHE:
        _CACHE["nc"] = build_program()
    nc = _CACHE["nc"]

    in_maps = prep_in_maps(inputs)
    res = run_bass_kernel_spmd(
        nc, in_maps, core_ids=list(range(NCORES)),
        **_CACHE.get("run_kwargs", {}),
    )
    _CACHE["last_results"] = res

    logits = np.concatenate([r["logits"] for r in res.results], axis=0)
    feat = np.concatenate([r["feat"] for r in res.results], axis=0)
    values = np.concatenate([r["values"] for r in res.results], axis=0)[:, 0]
    labels = np.concatenate([r["labels"] for r in res.results], axis=0)[:, 0]
    return logits, feat, values, labels.astype(np.int32)


# revision 20
# speedup vs baseline: 1.0185x; 1.0185x over previous
"""Trainium2 Bass kernel for nn_MetaEmbedding_Classifier (retrieval_knn).

Data-parallel over batch B=1024 across 8 NeuronCores (128 rows/core).
All relation/centroid/classifier weights replicated per core.

Per-core pipeline (all math on device; host only slices / transposes /
pads layouts):
  feat  = concat(head, tail) @ fc2_w.T + fc2_b          (PE, K-chunked)
  featT = feat^T (PE transposes, stationary operand for later matmuls)
  negd2 = 2*(x @ centT  +  ones (x) (-c2/2)) - |x|^2    (PE + DVE)
  values/labels = max / max_index over negd2             (DVE top-8 ops)
  att   = softmax(x @ all_relation.T) masked at `relation`
  mf    = att^T.T @ centroids                            (PE)
  p     = sigmoid(x @ sel_w.T + sel_b)
  fused = (mf - feat)*p + feat
  logits = (ex @ cls_w.T) * (SCALE/||cls_w||)  with ex = fused/(1+|fused|)

K-chunked operands are packed host-side into [128, nchunk*N] so every DMA
is a single per-partition-contiguous transfer.
"""

from contextlib import ExitStack

import numpy as np

import concourse.bass as bass
import concourse.mybir as mybir
import concourse.tile as tile
from concourse import bacc
from concourse.bass_utils import run_bass_kernel_spmd

B, D, R = 1024, 512, 500
NCORES = 8
BC = B // NCORES  # 128 batch rows per core
KD = D // 128     # 4 k-chunks over D
K2D = 2 * D // 128  # 8 k-chunks over 2D
RCH = 125         # att/cent r-chunk (4 * 125 = 500)
SCALE = 16.0
EPS = 1e-12

F32 = mybir.dt.float32
F32R = mybir.dt.float32r
I32 = mybir.dt.int32
U32 = mybir.dt.uint32

# f32r = fp32 matmul mode that streams 1 row/cycle (vs 4 for plain fp32)
# when the moving free dim >= 256.
USE_F32R = False


def _r(ap):
    """AP used as a matmul operand: bitcast to float32r for 4x PE rate."""
    return ap.bitcast(F32R) if USE_F32R else ap


def _pack(mat, p):
    """[K*p, N] -> [p, K*N] so chunk k is columns k*N:(k+1)*N."""
    kp, n = mat.shape
    k = kp // p
    return np.ascontiguousarray(
        mat.reshape(k, p, n).transpose(1, 0, 2).reshape(p, k * n)
    )


def build_program():
    nc = bacc.Bacc(target_bir_lowering=False, trn_type="TRN2", debug=False)

    din = {}

    def inp(name, shape):
        din[name] = nc.dram_tensor(name, shape, F32, kind="ExternalInput").ap()

    inp("xT", [128, K2D * BC])       # concat(head,tail)^T packed, per core
    inp("w2T", [128, K2D * D])       # fc2_w.T packed (replicated)
    inp("b2row", [1, D])             # fc2_b row
    inp("centT", [128, KD * R])      # centroids[3:].T zero-padded, packed
    inp("cent", [RCH, 4 * D])        # centroids row-major, packed in 4 r-chunks
    inp("relT", [128, KD * R])       # all_relation.T packed
    inp("clsT", [128, KD * R])       # cls_w.T packed
    inp("selT", [128, KD])           # sel_w.T packed
    inp("selb", [1, 1])
    inp("iota", [1, R])              # arange(500) as f32
    inp("relf", [BC, 1])             # relation indices as f32, per core
    inp("ident", [128, 128])         # identity for PE transposes

    dout = {
        "logits": nc.dram_tensor("logits", [BC, R], F32,
                                 kind="ExternalOutput").ap(),
        "feat": nc.dram_tensor("feat", [BC, D], F32,
                               kind="ExternalOutput").ap(),
        "values": nc.dram_tensor("values", [BC, 1], F32,
                                 kind="ExternalOutput").ap(),
        "labels": nc.dram_tensor("labels", [BC, 1], I32,
                                 kind="ExternalOutput").ap(),
    }

    with tile.TileContext(nc) as tc:
        with ExitStack() as ctx:
            _build(ctx, tc, din, dout)

    nc.compile()
    return nc


def _build(ctx, tc, din, dout):
    nc = tc.nc
    Exp = mybir.ActivationFunctionType.Exp
    Sqrt = mybir.ActivationFunctionType.Sqrt
    Square = mybir.ActivationFunctionType.Square
    Sigmoid = mybir.ActivationFunctionType.Sigmoid
    AX = mybir.AxisListType.X
    Op = mybir.AluOpType

    singles = ctx.enter_context(tc.tile_pool(name="singles", bufs=1))
    work = ctx.enter_context(tc.tile_pool(name="work", bufs=2))
    psmm = ctx.enter_context(tc.tile_pool(name="psmm", bufs=3, space="PSUM"))
    pstr = ctx.enter_context(tc.tile_pool(name="pstr", bufs=2, space="PSUM"))
    psrow = ctx.enter_context(tc.tile_pool(name="psrow", bufs=1, space="PSUM"))

    def load(name, shape, pool=singles, eng=None):
        t = pool.tile(shape, F32, tag=name, name=name + "_sb")
        (eng or nc.sync).dma_start(out=t[:], in_=din[name])
        return t

    # ---- resident constants / weights, spread across the three DMA rings
    # (sync / scalar / gpsimd) in consumption order: transfers are FIFO per
    # ring at ~125GB/s each, so a single ring serializes everything.
    xT_f = load("xT", [128, K2D * BC])                      # sync: fc2 first
    w2T_f = load("w2T", [128, K2D * D])
    centT_f = load("centT", [128, KD * R], eng=nc.scalar)   # dist next
    relT_f = load("relT", [128, KD * R], eng=nc.scalar)     # then attention
    ident = load("ident", [128, 128], eng=nc.gpsimd)
    iota = load("iota", [1, R], eng=nc.gpsimd)
    b2row = load("b2row", [1, D], eng=nc.gpsimd)
    selb = load("selb", [1, 1], eng=nc.gpsimd)
    relf = load("relf", [BC, 1], eng=nc.gpsimd)
    selT_f = load("selT", [128, KD], eng=nc.gpsimd)
    cent_f = load("cent", [RCH, 4 * D], eng=nc.gpsimd)      # mf mid-kernel
    clsT_f = load("clsT", [128, KD * R], eng=nc.scalar)     # classifier last

    xT = [xT_f[:, k * BC:(k + 1) * BC] for k in range(K2D)]
    w2T = [w2T_f[:, k * D:(k + 1) * D] for k in range(K2D)]
    centT = [centT_f[:, k * R:(k + 1) * R] for k in range(KD)]
    relT = [relT_f[:, k * R:(k + 1) * R] for k in range(KD)]
    clsT = [clsT_f[:, k * R:(k + 1) * R] for k in range(KD)]
    selT = [selT_f[:, k:k + 1] for k in range(KD)]
    cent = [cent_f[:, j * D:(j + 1) * D] for j in range(4)]

    ones = singles.tile([128, 128], F32, tag="ones", name="ones")
    nc.vector.memset(ones[:], 1.0)
    ones_row = ones[0:1, :]   # [1, 128] lhsT for K=1 row-broadcast matmuls
    ones_col = ones[:, 0:1]   # [128, 1] lhsT for partition-sum matmuls

    # ================= fc2: feat = concat @ fc2_w.T + b =================
    ps_feat = psmm.tile([BC, D], F32, tag="mm", name="ps_feat")
    for k in range(K2D):
        nc.tensor.matmul(ps_feat[:], _r(xT[k]), _r(w2T[k]), start=(k == 0),
                         stop=False)
    nc.tensor.matmul(ps_feat[:], _r(ones_row), _r(b2row[:]), start=False,
                     stop=True)

    feat = singles.tile([BC, D], F32, tag="feat", name="feat")
    nc.vector.tensor_copy(feat[:], ps_feat[:])
    nc.gpsimd.dma_start(out=dout["feat"], in_=feat[:])

    # |x|^2 per row (ACT square w/ fused free-dim accumulation)
    x2 = singles.tile([BC, 1], F32, tag="x2", name="x2")
    sq_d = work.tile([BC, D], F32, tag="sq_d", name="sq_d0")
    nc.scalar.activation(sq_d[:], ps_feat[:], Square, accum_out=x2[:])

    # featT chunks [128, BC] (stationary operand for x @ W.T style matmuls)
    featT = []
    for k in range(KD):
        pt = pstr.tile([128, BC], F32, tag="tr", name=f"ps_ft{k}")
        nc.tensor.transpose(pt[:], feat[:, k * 128:(k + 1) * 128], ident[:])
        st = singles.tile([128, BC], F32, tag=f"featT{k}", name=f"featT{k}")
        nc.vector.tensor_copy(st[:], pt[:])
        featT.append(st)

    # ================= distances to centroids[3:] =================
    # c2m = -0.5 * sum_d centT^2 (per padded column; pad cols give 0)
    ps_c2 = psrow.tile([1, R], F32, tag="row", name="ps_c2")
    for k in range(KD):
        sq = work.tile([128, R], F32, tag="sq_r", name=f"sq_c{k}")
        nc.scalar.activation(sq[:], centT[k], Square)
        nc.tensor.matmul(ps_c2[:], _r(ones_col), _r(sq[:]), start=(k == 0),
                         stop=(k == KD - 1))
    c2m = singles.tile([1, R], F32, tag="c2m", name="c2m")
    nc.vector.tensor_scalar(c2m[:], ps_c2[:], -0.5, None, op0=Op.mult)

    # G = x @ centT - c2/2  ->  negd2 = 2G - |x|^2
    ps_g = psmm.tile([BC, R], F32, tag="mm", name="ps_g")
    for k in range(KD):
        nc.tensor.matmul(ps_g[:], _r(featT[k][:]), _r(centT[k]), start=(k == 0),
                         stop=False)
    nc.tensor.matmul(ps_g[:], _r(ones_row), _r(c2m[:]), start=False, stop=True)

    negd2 = work.tile([BC, R], F32, tag="negd2", name="negd2")
    nc.vector.tensor_scalar(negd2[:], ps_g[:], 2.0, x2[:], op0=Op.mult,
                            op1=Op.subtract)

    # top-1 via DVE top-8 ops over the 497 valid columns
    maxv = work.tile([BC, 8], F32, tag="maxv", name="maxv")
    nc.vector.max(maxv[:], negd2[:, 0:R - 3])
    idx = work.tile([BC, 8], U32, tag="idx", name="idx")
    nc.vector.max_index(idx[:], maxv[:], negd2[:, 0:R - 3])

    vtmp = work.tile([BC, 1], F32, tag="vtmp", name="vtmp")
    nc.vector.tensor_scalar(vtmp[:], maxv[:, 0:1], -1.0, EPS, op0=Op.mult,
                            op1=Op.max)
    values = work.tile([BC, 1], F32, tag="values", name="values_sb")
    nc.scalar.activation(values[:], vtmp[:], Sqrt)
    nc.gpsimd.dma_start(out=dout["values"], in_=values[:])

    idxf = work.tile([BC, 1], F32, tag="idxf", name="idxf")
    nc.vector.tensor_copy(idxf[:], idx[:, 0:1])
    labels = work.tile([BC, 1], I32, tag="labels", name="labels_sb")
    nc.vector.tensor_scalar(labels[:], idxf[:], 3.0, None, op0=Op.add)
    nc.gpsimd.dma_start(out=dout["labels"], in_=labels[:])

    # ================= masked softmax attention =================
    ps_s = psmm.tile([BC, R], F32, tag="mm", name="ps_s")
    for k in range(KD):
        nc.tensor.matmul(ps_s[:], _r(featT[k][:]), _r(relT[k]), start=(k == 0),
                         stop=(k == KD - 1))

    smax = work.tile([BC, 1], F32, tag="smax", name="smax")
    nc.vector.tensor_reduce(smax[:], ps_s[:], axis=AX, op=Op.max)
    nsmax = work.tile([BC, 1], F32, tag="nsmax", name="nsmax")
    nc.vector.tensor_scalar(nsmax[:], smax[:], -1.0, None, op0=Op.mult)

    e = work.tile([BC, R], F32, tag="e", name="e")
    ssum = work.tile([BC, 1], F32, tag="ssum", name="ssum")
    nc.scalar.activation(e[:], ps_s[:], Exp, bias=nsmax[:], accum_out=ssum[:])
    rinv = work.tile([BC, 1], F32, tag="rinv", name="rinv")
    nc.vector.reciprocal(rinv[:], ssum[:])

    # mask = (iota != relation); iota broadcast across partitions via K=1 matmul
    ps_i = psmm.tile([BC, R], F32, tag="mm", name="ps_i")
    nc.tensor.matmul(ps_i[:], _r(ones_row), _r(iota[:]), start=True, stop=True)
    mask = work.tile([BC, R], F32, tag="mask", name="mask")
    nc.vector.tensor_scalar(mask[:], ps_i[:], relf[:], None, op0=Op.not_equal)

    am = work.tile([BC, R], F32, tag="am", name="am")
    nc.vector.scalar_tensor_tensor(am[:], e[:], rinv[:], mask[:], op0=Op.mult,
                                   op1=Op.mult)

    # attT chunks [125, BC]; mf = att @ centroids
    ps_mf = psmm.tile([BC, D], F32, tag="mm", name="ps_mf")
    for j in range(4):
        pt = pstr.tile([RCH, BC], F32, tag="tr", name=f"ps_at{j}")
        nc.tensor.transpose(pt[:], am[:, j * RCH:(j + 1) * RCH], ident[:])
        attT = work.tile([RCH, BC], F32, tag=f"attT{j}", name=f"attT{j}")
        nc.vector.tensor_copy(attT[:], pt[:])
        nc.tensor.matmul(ps_mf[:], _r(attT[:]), _r(cent[j]), start=(j == 0),
                         stop=(j == 3))

    # ================= gate + fuse + cosnorm classifier =================
    ps_sel = psmm.tile([BC, 1], F32, tag="sel", name="ps_sel", bufs=1)
    for k in range(KD):
        nc.tensor.matmul(ps_sel[:], _r(featT[k][:]), _r(selT[k]), start=(k == 0),
                         stop=False)
    nc.tensor.matmul(ps_sel[:], _r(ones_row), _r(selb[:]), start=False,
                     stop=True)
    p = work.tile([BC, 1], F32, tag="p", name="p")
    nc.scalar.activation(p[:], ps_sel[:], Sigmoid)

    # fused = (mf - feat) * p + feat
    dmf = work.tile([BC, D], F32, tag="dmf", name="dmf")
    nc.vector.tensor_tensor(dmf[:], ps_mf[:], feat[:], op=Op.subtract)
    fused = work.tile([BC, D], F32, tag="fused", name="fused")
    nc.vector.scalar_tensor_tensor(fused[:], dmf[:], p[:], feat[:], op0=Op.mult,
                                   op1=Op.add)

    # ex = fused / (1 + |fused|)
    fn2 = work.tile([BC, 1], F32, tag="fn2", name="fn2")
    sq_f = work.tile([BC, D], F32, tag="sq_d", name="sq_d1")
    nc.scalar.activation(sq_f[:], fused[:], Square, accum_out=fn2[:])
    fn = work.tile([BC, 1], F32, tag="fn", name="fn")
    nc.scalar.activation(fn[:], fn2[:], Sqrt)
    fn1 = work.tile([BC, 1], F32, tag="fn1", name="fn1")
    nc.vector.tensor_scalar(fn1[:], fn[:], 1.0, None, op0=Op.add)
    s = work.tile([BC, 1], F32, tag="s", name="s")
    nc.vector.reciprocal(s[:], fn1[:])
    ex = work.tile([BC, D], F32, tag="ex", name="ex")
    nc.vector.tensor_scalar(ex[:], fused[:], s[:], None, op0=Op.mult)

    exT = []
    for k in range(KD):
        pt = pstr.tile([128, BC], F32, tag="tr", name=f"ps_xt{k}")
        nc.tensor.transpose(pt[:], ex[:, k * 128:(k + 1) * 128], ident[:])
        st = work.tile([128, BC], F32, tag=f"exT{k}", name=f"exT{k}")
        nc.vector.tensor_copy(st[:], pt[:])
        exT.append(st)

    # rn = SCALE / ||cls_w|| per class (row vector)
    ps_n2 = psrow.tile([1, R], F32, tag="row", name="ps_n2")
    for k in range(KD):
        sq = work.tile([128, R], F32, tag="sq_r", name=f"sq_n{k}")
        nc.scalar.activation(sq[:], clsT[k], Square)
        nc.tensor.matmul(ps_n2[:], _r(ones_col), _r(sq[:]), start=(k == 0),
                         stop=(k == KD - 1))
    nrm = work.tile([1, R], F32, tag="nrm", name="nrm")
    nc.scalar.activation(nrm[:], ps_n2[:], Sqrt)
    rn0 = work.tile([1, R], F32, tag="rn0", name="rn0")
    nc.vector.reciprocal(rn0[:], nrm[:])
    rn = singles.tile([1, R], F32, tag="rn", name="rn")
    nc.vector.tensor_scalar(rn[:], rn0[:], SCALE, None, op0=Op.mult)

    # logits = (ex @ clsT) * rn  (rn broadcast via K=1 matmul)
    ps_g2 = psmm.tile([BC, R], F32, tag="mm", name="ps_g2")
    for k in range(KD):
        nc.tensor.matmul(ps_g2[:], _r(exT[k][:]), _r(clsT[k]), start=(k == 0),
                         stop=(k == KD - 1))
    ps_rb = psmm.tile([BC, R], F32, tag="mm", name="ps_rb")
    nc.tensor.matmul(ps_rb[:], _r(ones_row), _r(rn[:]), start=True, stop=True)
    rb = work.tile([BC, R], F32, tag="rb", name="rb")
    nc.vector.tensor_copy(rb[:], ps_rb[:])
    logits = work.tile([BC, R], F32, tag="logits", name="logits_sb")
    nc.vector.tensor_tensor(logits[:], ps_g2[:], rb[:], op=Op.mult)
    nc.sync.dma_start(out=dout["logits"], in_=logits[:])


# ---------------------------------------------------------------------------
# host-side entry point
# ---------------------------------------------------------------------------

_CACHE = {}


def prep_in_maps(inputs):
    head = np.asarray(inputs["head_entity"], np.float32)
    tail = np.asarray(inputs["tail_entity"], np.float32)
    relation = np.asarray(inputs["relation"])
    all_rel = np.asarray(inputs["all_relation"], np.float32)
    centroids = np.asarray(inputs["centroids"], np.float32)
    fc2_w = np.asarray(inputs["fc2_w"], np.float32)
    fc2_b = np.asarray(inputs["fc2_b"], np.float32)
    sel_w = np.asarray(inputs["sel_w"], np.float32)
    sel_b = np.asarray(inputs["sel_b"], np.float32)
    cls_w = np.asarray(inputs["cls_w"], np.float32)

    concat = np.concatenate([head, tail], axis=1)          # [B, 2D]
    centT_pad = np.zeros((D, R), np.float32)
    centT_pad[:, : R - 3] = centroids[3:].T

    shared = {
        "w2T": _pack(np.ascontiguousarray(fc2_w.T), 128),  # [128, 8*D]
        "b2row": fc2_b.reshape(1, D),
        "centT": _pack(centT_pad, 128),
        "cent": _pack(centroids, RCH),
        "relT": _pack(np.ascontiguousarray(all_rel.T), 128),
        "clsT": _pack(np.ascontiguousarray(cls_w.T), 128),
        "selT": _pack(np.ascontiguousarray(sel_w.T), 128),
        "selb": sel_b.reshape(1, 1).astype(np.float32),
        "iota": np.arange(R, dtype=np.float32).reshape(1, R),
        "ident": np.eye(128, dtype=np.float32),
    }
    in_maps = []
    for c in range(NCORES):
        sl = slice(c * BC, (c + 1) * BC)
        m = dict(shared)
        m["xT"] = _pack(np.ascontiguousarray(concat[sl].T), 128)
        m["relf"] = relation[sl].astype(np.float32).reshape(BC, 1)
        in_maps.append(m)
    return in_maps


def kernel(**inputs):
    if "nc" not in _CACHE:
        _CACHE["nc"] = build_program()
    nc = _CACHE["nc"]

    in_maps = prep_in_maps(inputs)
    res = run_bass_kernel_spmd(
        nc, in_maps, core_ids=list(range(NCORES)),
        **_CACHE.get("run_kwargs", {}),
    )
    _CACHE["last_results"] = res

    logits = np.concatenate([r["logits"] for r in res.results], axis=0)
    feat = np.concatenate([r["feat"] for r in res.results], axis=0)
    values = np.concatenate([r["values"] for r in res.results], axis=0)[:, 0]
    labels = np.concatenate([r["labels"] for r in res.results], axis=0)[:, 0]
    return logits, feat, values, labels.astype(np.int32)
